# revision 4
# baseline (speedup 1.0000x reference)
"""PickNMSPredictionsAndReturnAsBatchedResult on 8 Trainium2 NeuronCores.

Data-parallel over the batch dim: core c owns images [8c, 8c+8).  The host
partitions selected_indexes rows by batch (order preserving) and pads each
core's row list to SROWS = 128*NCH.  Row r lives at [partition r%128,
column r//128] of the on-chip [128, NCH] index tiles.

On device each core:
  1. gathers its rows' boxes (16B) and scores (4B) from HBM via indirect
     DMA, one 128-row chunk per instruction (HW consumes one index per
     partition and moves the dest free extent contiguously)
  2. computes per-image stable ranks: within-chunk prefix over partitions
     via an inclusive lower-triangular matmul, cross-chunk prefix via tiny
     free-dim scans, broadcast back over partitions with a K=1 matmul
  3. scatters [x1,y1,x2,y2,score,class] 24B rows into a [8*300, 6] output
     via indirect DMA; padded/overflow rows get slot >= 2400 and are
     dropped by bounds_check.
Outputs are pre-zeroed by the runtime, so untouched slots stay 0.
"""

import numpy as np

B = 64
N = 8192
C = 91
MAX_PRED = 300
NCORES = 8
BLOC = B // NCORES  # images per core
P = 128
NCH_DEFAULT = 16  # chunks of 128 rows per core

_CACHE = {}


def _build(nch):
    from concourse import bacc, bass, mybir, tile

    f32 = mybir.dt.float32
    i32 = mybir.dt.int32
    AL = mybir.AluOpType
    BIG = 1 << 20
    NSLOT = BLOC * MAX_PRED

    nc = bacc.Bacc(
        "TRN2",
        target_bir_lowering=False,
        debug=False,
        enable_asserts=False,
        num_devices=NCORES,
    )

    boxes = nc.dram_tensor("boxes", [BLOC * N, 4], f32, kind="ExternalInput")
    scores = nc.dram_tensor("scores", [BLOC * N * C, 1], f32, kind="ExternalInput")
    sidx = nc.dram_tensor("sidx", [P, 4 * nch], i32, kind="ExternalInput")
    num_pred = nc.dram_tensor("num_pred", [BLOC, 1], i32, kind="ExternalOutput")
    out_rows = nc.dram_tensor("out_rows", [NSLOT, 6], f32, kind="ExternalOutput")

    with tile.TileContext(nc) as tc:
        with (
            tc.tile_pool(name="sb", bufs=1) as sb,
            tc.tile_pool(name="ps", bufs=1, space="PSUM") as ps,
        ):
            sidx_sb = sb.tile([P, 4 * nch], i32)
            nc.sync.dma_start(sidx_sb[:], sidx.ap())

            b_i = sidx_sb[:, 0 * nch : 1 * nch]  # batch-local id, BLOC for pads
            lab_i = sidx_sb[:, 1 * nch : 2 * nch]  # class id
            box_i = sidx_sb[:, 2 * nch : 3 * nch]  # box id
            val_i = sidx_sb[:, 3 * nch : 4 * nch]  # 1 for real rows

            # ---- gather offsets (int32) ----
            bg = sb.tile([P, nch], i32)
            nc.vector.tensor_tensor(out=bg[:], in0=b_i, in1=val_i, op=AL.mult)
            gbox = sb.tile([P, nch], i32)
            nc.vector.tensor_scalar(
                out=gbox[:], in0=bg[:], scalar1=N, scalar2=None, op0=AL.mult
            )
            nc.vector.tensor_tensor(out=gbox[:], in0=gbox[:], in1=box_i, op=AL.add)
            gsc = sb.tile([P, nch], i32)
            nc.vector.tensor_scalar(
                out=gsc[:], in0=gbox[:], scalar1=C, scalar2=None, op0=AL.mult
            )
            nc.vector.tensor_tensor(out=gsc[:], in0=gsc[:], in1=lab_i, op=AL.add)

            # ---- gathers from HBM: one 128-row chunk per instruction ----
            boxg = sb.tile([P, 4 * nch], f32)
            scog = sb.tile([P, nch], f32)
            for j in range(nch):
                nc.gpsimd.indirect_dma_start(
                    out=boxg[:, 4 * j : 4 * j + 4],
                    out_offset=None,
                    in_=boxes.ap(),
                    in_offset=bass.IndirectOffsetOnAxis(ap=gbox[:, j : j + 1], axis=0),
                )
            for j in range(nch):
                nc.gpsimd.indirect_dma_start(
                    out=scog[:, j : j + 1],
                    out_offset=None,
                    in_=scores.ap(),
                    in_offset=bass.IndirectOffsetOnAxis(ap=gsc[:, j : j + 1], axis=0),
                )

            # ---- one-hot over local batch (c-major free: [P, BLOC, nch]) ----
            bf = sb.tile([P, nch], f32)
            nc.vector.tensor_copy(out=bf[:], in_=b_i)
            oh = sb.tile([P, BLOC * nch], f32)
            for c in range(BLOC):
                nc.vector.tensor_scalar(
                    out=oh[:, c * nch : (c + 1) * nch],
                    in0=bf[:],
                    scalar1=float(c),
                    scalar2=None,
                    op0=AL.is_equal,
                )

            # ---- within-chunk inclusive prefix over partitions (matmul) ----
            ones = sb.tile([P, P], f32)
            nc.gpsimd.memset(ones[:], 1.0)
            lti = sb.tile([P, P], f32)
            # keep ones where (i - j) >= 0, i.e. inclusive lower in (contraction j, out i)
            nc.gpsimd.affine_select(
                out=lti[:],
                in_=ones[:],
                pattern=[[1, P]],
                base=0,
                channel_multiplier=-1,
                compare_op=AL.is_ge,
                fill=0.0,
            )
            cum = ps.tile([P, BLOC * nch], f32)
            nc.tensor.matmul(out=cum[:], lhsT=lti[:], rhs=oh[:], start=True, stop=False)

            # ---- chunk totals on partition 0: tot[c*nch+j] = sum_p oh[p, .] ----
            totp = ps.tile([1, BLOC * nch], f32)
            nc.tensor.matmul(
                out=totp[:], lhsT=ones[:, 0:1], rhs=oh[:], start=True, stop=True
            )
            tot = sb.tile([1, BLOC * nch], f32)
            nc.vector.tensor_copy(out=tot[:], in_=totp[:])

            # ---- cross-chunk exclusive prefix: incl scan - tot ----
            incl = sb.tile([1, BLOC * nch], f32)
            for c in range(BLOC):
                s = slice(c * nch, (c + 1) * nch)
                nc.vector.tensor_tensor_scan(
                    out=incl[:, s],
                    data0=tot[:, s],
                    data1=tot[:, s],
                    initial=0.0,
                    op0=AL.add,
                    op1=AL.bypass,
                )
            pref = sb.tile([1, BLOC * nch], f32)
            nc.vector.tensor_tensor(
                out=pref[:], in0=incl[:], in1=tot[:], op=AL.subtract
            )

            # ---- broadcast pref over partitions into cum (K=1 matmul) ----
            nc.tensor.matmul(
                out=cum[:], lhsT=ones[0:1, :], rhs=pref[:], start=False, stop=True
            )

            # ---- rank of each row inside its own image ----
            vs = sb.tile([P, BLOC * nch], f32)
            nc.vector.tensor_tensor(out=vs[:], in0=cum[:], in1=oh[:], op=AL.mult)
            rankf = sb.tile([P, nch], f32)
            nc.vector.tensor_reduce(
                out=rankf[:],
                in_=vs[:].rearrange("p (c j) -> p j c", j=nch),
                axis=mybir.AxisListType.X,
                op=AL.add,
            )

            # ---- scatter slot = b*300 + rank - 1, drops pushed OOB ----
            rank0 = sb.tile([P, nch], i32)
            nc.vector.tensor_scalar(
                out=rank0[:], in0=rankf[:], scalar1=-1.0, scalar2=None, op0=AL.add
            )
            slot = sb.tile([P, nch], i32)
            nc.vector.tensor_scalar(
                out=slot[:], in0=bg[:], scalar1=MAX_PRED, scalar2=None, op0=AL.mult
            )
            nc.vector.tensor_tensor(out=slot[:], in0=slot[:], in1=rank0[:], op=AL.add)
            ge = sb.tile([P, nch], i32)
            nc.vector.tensor_scalar(
                out=ge[:],
                in0=rankf[:],
                scalar1=float(MAX_PRED) + 0.5,
                scalar2=float(BIG),
                op0=AL.is_ge,
                op1=AL.mult,
            )
            iv = sb.tile([P, nch], i32)
            nc.vector.tensor_scalar(
                out=iv[:],
                in0=val_i,
                scalar1=0,
                scalar2=BIG,
                op0=AL.is_equal,
                op1=AL.mult,
            )
            nc.vector.tensor_tensor(out=slot[:], in0=slot[:], in1=ge[:], op=AL.add)
            nc.vector.tensor_tensor(out=slot[:], in0=slot[:], in1=iv[:], op=AL.add)

            # ---- assemble 6-wide rows: [x1 y1 x2 y2 score class] ----
            rows = sb.tile([P, 6 * nch], f32)
            rows3 = rows[:].rearrange("p (j s) -> p j s", s=6)
            nc.vector.tensor_copy(
                out=rows3[:, :, 0:4],
                in_=boxg[:].rearrange("p (j s) -> p j s", s=4),
            )
            nc.vector.tensor_copy(
                out=rows3[:, :, 4:5],
                in_=scog[:].rearrange("p (j s) -> p j s", s=1),
            )
            nc.vector.tensor_copy(
                out=rows3[:, :, 5:6],
                in_=lab_i.rearrange("p (j s) -> p j s", s=1),
            )

            # ---- scatter rows, one 128-row chunk per instruction ----
            for j in range(nch):
                nc.gpsimd.indirect_dma_start(
                    out=out_rows.ap(),
                    out_offset=bass.IndirectOffsetOnAxis(ap=slot[:, j : j + 1], axis=0),
                    in_=rows[:, 6 * j : 6 * j + 6],
                    in_offset=None,
                    bounds_check=NSLOT - 1,
                    oob_is_err=False,
                )

            # ---- per-image counts: reduce tot over chunks ----
            cntf = sb.tile([1, BLOC], f32)
            nc.vector.tensor_reduce(
                out=cntf[:],
                in_=tot[:].rearrange("p (c j) -> p c j", j=nch),
                axis=mybir.AxisListType.X,
                op=AL.add,
            )
            cnti = sb.tile([1, BLOC], i32)
            nc.vector.tensor_copy(out=cnti[:], in_=cntf[:])
            nc.sync.dma_start(num_pred.ap().rearrange("b one -> one b"), cnti[:])

    nc.compile()
    return nc


def get_nc(nch=NCH_DEFAULT):
    if nch not in _CACHE:
        _CACHE[nch] = _build(nch)
    return _CACHE[nch]


def _pack(arr, nch):
    """[nch*128] row-ordered -> [128, nch] with element [p, j] = row j*128+p."""
    return np.ascontiguousarray(arr.reshape(nch, P).T)


def shard_inputs(pred_boxes, pred_scores, selected_indexes, nch):
    """Build the 8 per-core input maps from the full inputs."""
    pred_boxes = np.asarray(pred_boxes, dtype=np.float32)
    pred_scores = np.asarray(pred_scores, dtype=np.float32)
    sel = np.asarray(selected_indexes).astype(np.int64)
    srows = nch * P

    b_all = sel[:, 0]
    in_maps = []
    for core in range(NCORES):
        rows = np.nonzero((b_all >= core * BLOC) & (b_all < (core + 1) * BLOC))[0]
        cnt = len(rows)
        assert cnt <= srows, f"core {core} has {cnt} rows > {srows}"
        bs = np.full(srows, BLOC, np.int32)
        labp = np.zeros(srows, np.int32)
        boxp = np.zeros(srows, np.int32)
        vld = np.zeros(srows, np.int32)
        bs[:cnt] = (sel[rows, 0] - core * BLOC).astype(np.int32)
        labp[:cnt] = sel[rows, 1].astype(np.int32)
        boxp[:cnt] = sel[rows, 2].astype(np.int32)
        vld[:cnt] = 1
        sidx = np.concatenate(
            [_pack(bs, nch), _pack(labp, nch), _pack(boxp, nch), _pack(vld, nch)],
            axis=1,
        )  # [P, 4*nch]
        in_maps.append(
            {
                "boxes": np.ascontiguousarray(
                    pred_boxes[core * BLOC : (core + 1) * BLOC]
                ).reshape(BLOC * N, 4),
                "scores": np.ascontiguousarray(
                    pred_scores[core * BLOC : (core + 1) * BLOC]
                ).reshape(BLOC * N * C, 1),
                "sidx": np.ascontiguousarray(sidx),
            }
        )
    return in_maps


def unshard_outputs(results):
    """results: list of 8 dicts name->array. Returns the reference tuple."""
    num_pred = np.concatenate([r["num_pred"] for r in results], axis=0).astype(
        np.int32
    )  # [64, 1]
    rows = np.stack([r["out_rows"] for r in results], axis=0)  # [8, 2400, 6]
    rows = rows.reshape(B, MAX_PRED, 6)
    out_boxes = np.ascontiguousarray(rows[:, :, 0:4], dtype=np.float32)
    out_scores = np.ascontiguousarray(rows[:, :, 4], dtype=np.float32)
    out_classes = rows[:, :, 5].astype(np.int32)
    return num_pred, out_boxes, out_scores, out_classes


def _pick_nch(selected_indexes):
    sel = np.asarray(selected_indexes).astype(np.int64)
    counts = np.bincount(sel[:, 0] // BLOC, minlength=NCORES)
    need = int(counts.max())
    nch = NCH_DEFAULT
    while nch * P < need:
        nch *= 2
    return nch


def kernel(pred_boxes, pred_scores, selected_indexes):
    from concourse import bass_utils

    nch = _pick_nch(selected_indexes)
    nc = get_nc(nch)
    in_maps = shard_inputs(pred_boxes, pred_scores, selected_indexes, nch)
    res = bass_utils.run_bass_kernel_spmd(nc, in_maps, core_ids=list(range(NCORES)))
    return unshard_outputs(res.results)


# revision 7
# speedup vs baseline: 1.9384x; 1.9384x over previous
"""PickNMSPredictionsAndReturnAsBatchedResult on 8 Trainium2 NeuronCores.

Data-parallel over the batch dim: core c owns images [8c, 8c+8).  The host
partitions selected_indexes rows by batch (order preserving), pads each
core's row list to SROWS = 128*NCH, and builds a combined per-core table
tbl[b, n] = [box(4) | scores(91)] so one indirect-DMA descriptor fetches a
row's box AND its 91 scores together.

On device each core:
  1. gathers 95-float rows from tbl via indirect DMA, one 128-row chunk per
     instruction (HW: one index per partition, contiguous run per index)
  2. selects each row's score via a 91-wide one-hot multiply-reduce
  3. computes per-image stable ranks: prefix over partitions via an
     inclusive lower-triangular matmul, cross-chunk prefix via tiny
     free-dim scans, broadcast back over partitions with a K=1 matmul
  4. scatters [x1,y1,x2,y2,score,class] 24B rows via indirect DMA into a
     per-chunk [2400, 6] output (separate outputs break the WAW chain);
     padded/overflow rows get slot >= 2400 and drop via bounds_check.
Outputs are pre-zeroed by the runtime; the host sums the per-chunk outputs
(disjoint slots) into the final [8*300, 6] block per core.
"""

import numpy as np

B = 64
N = 8192
C = 91
MAX_PRED = 300
NCORES = 8
BLOC = B // NCORES  # images per core
P = 128
NCH_DEFAULT = 9  # chunks of 128 rows per core
TW = 4 + C  # combined table row width: box(4) + scores(91)

_CACHE = {}


def _build(nch):
    from concourse import bacc, bass, mybir, tile

    f32 = mybir.dt.float32
    i32 = mybir.dt.int32
    AL = mybir.AluOpType
    BIG = 1 << 20
    NSLOT = BLOC * MAX_PRED

    nc = bacc.Bacc(
        "TRN2",
        target_bir_lowering=False,
        debug=False,
        enable_asserts=False,
        num_devices=NCORES,
    )

    tbl = nc.dram_tensor("tbl", [BLOC * N, TW], f32, kind="ExternalInput")
    sidx = nc.dram_tensor("sidx", [P, 4 * nch], i32, kind="ExternalInput")
    num_pred = nc.dram_tensor("num_pred", [BLOC, 1], i32, kind="ExternalOutput")
    outs = [
        nc.dram_tensor(f"out_rows_{j}", [NSLOT, 6], f32, kind="ExternalOutput")
        for j in range(nch)
    ]

    with tile.TileContext(nc) as tc:
        with (
            tc.tile_pool(name="sb", bufs=1) as sb,
            tc.tile_pool(name="ps", bufs=1, space="PSUM") as ps,
        ):
            sidx_sb = sb.tile([P, 4 * nch], i32)
            nc.sync.dma_start(sidx_sb[:], sidx.ap())

            b_i = sidx_sb[:, 0 * nch : 1 * nch]  # batch-local id, BLOC for pads
            lab_i = sidx_sb[:, 1 * nch : 2 * nch]  # class id
            box_i = sidx_sb[:, 2 * nch : 3 * nch]  # box id
            val_i = sidx_sb[:, 3 * nch : 4 * nch]  # 1 for real rows

            # ---- gather row index (int32) ----
            bg = sb.tile([P, nch], i32)
            nc.vector.tensor_tensor(out=bg[:], in0=b_i, in1=val_i, op=AL.mult)
            grow = sb.tile([P, nch], i32)
            nc.vector.tensor_scalar(
                out=grow[:], in0=bg[:], scalar1=N, scalar2=None, op0=AL.mult
            )
            nc.vector.tensor_tensor(out=grow[:], in0=grow[:], in1=box_i, op=AL.add)

            # ---- gathers: one 128-row chunk of 95-float rows per instruction ----
            gath = sb.tile([P, TW * nch], f32)
            for j in range(nch):
                nc.gpsimd.indirect_dma_start(
                    out=gath[:, TW * j : TW * (j + 1)],
                    out_offset=None,
                    in_=tbl.ap(),
                    in_offset=bass.IndirectOffsetOnAxis(ap=grow[:, j : j + 1], axis=0),
                )

            # ---- select each row's score: one-hot(lab) . scores ----
            i91 = sb.tile([P, C], i32)
            nc.gpsimd.iota(out=i91[:], pattern=[[1, C]], base=0, channel_multiplier=0)
            i91f = sb.tile([P, C], f32)
            nc.vector.tensor_copy(out=i91f[:], in_=i91[:])
            labf = sb.tile([P, nch], f32)
            nc.vector.tensor_copy(out=labf[:], in_=lab_i)
            oh91 = sb.tile([P, C * nch], f32)
            for j in range(nch):
                nc.vector.tensor_scalar(
                    out=oh91[:, C * j : C * (j + 1)],
                    in0=i91f[:],
                    scalar1=labf[:, j : j + 1],
                    scalar2=None,
                    op0=AL.is_equal,
                )
            vsco = sb.tile([P, C * nch], f32)
            nc.vector.tensor_tensor(
                out=vsco[:].rearrange("p (j s) -> p j s", s=C),
                in0=gath[:].rearrange("p (j s) -> p j s", s=TW)[:, :, 4:TW],
                in1=oh91[:].rearrange("p (j s) -> p j s", s=C),
                op=AL.mult,
            )
            sco = sb.tile([P, nch], f32)
            nc.vector.tensor_reduce(
                out=sco[:],
                in_=vsco[:].rearrange("p (j s) -> p j s", s=C),
                axis=mybir.AxisListType.X,
                op=AL.add,
            )

            # ---- one-hot over local batch (c-major free: [P, BLOC, nch]) ----
            bf = sb.tile([P, nch], f32)
            nc.vector.tensor_copy(out=bf[:], in_=b_i)
            oh = sb.tile([P, BLOC * nch], f32)
            for c in range(BLOC):
                nc.vector.tensor_scalar(
                    out=oh[:, c * nch : (c + 1) * nch],
                    in0=bf[:],
                    scalar1=float(c),
                    scalar2=None,
                    op0=AL.is_equal,
                )

            # ---- within-chunk inclusive prefix over partitions (matmul) ----
            ones = sb.tile([P, P], f32)
            nc.gpsimd.memset(ones[:], 1.0)
            lti = sb.tile([P, P], f32)
            # keep ones where (i - j) >= 0: inclusive lower in (contraction j, out i)
            nc.gpsimd.affine_select(
                out=lti[:],
                in_=ones[:],
                pattern=[[1, P]],
                base=0,
                channel_multiplier=-1,
                compare_op=AL.is_ge,
                fill=0.0,
            )
            cum = ps.tile([P, BLOC * nch], f32)
            nc.tensor.matmul(out=cum[:], lhsT=lti[:], rhs=oh[:], start=True, stop=False)

            # ---- chunk totals on partition 0 ----
            totp = ps.tile([1, BLOC * nch], f32)
            nc.tensor.matmul(
                out=totp[:], lhsT=ones[:, 0:1], rhs=oh[:], start=True, stop=True
            )
            tot = sb.tile([1, BLOC * nch], f32)
            nc.vector.tensor_copy(out=tot[:], in_=totp[:])

            # ---- cross-chunk exclusive prefix: incl scan - tot ----
            incl = sb.tile([1, BLOC * nch], f32)
            for c in range(BLOC):
                s = slice(c * nch, (c + 1) * nch)
                nc.vector.tensor_tensor_scan(
                    out=incl[:, s],
                    data0=tot[:, s],
                    data1=tot[:, s],
                    initial=0.0,
                    op0=AL.add,
                    op1=AL.bypass,
                )
            pref = sb.tile([1, BLOC * nch], f32)
            nc.vector.tensor_tensor(
                out=pref[:], in0=incl[:], in1=tot[:], op=AL.subtract
            )

            # ---- broadcast pref over partitions into cum (K=1 matmul) ----
            nc.tensor.matmul(
                out=cum[:], lhsT=ones[0:1, :], rhs=pref[:], start=False, stop=True
            )

            # ---- rank of each row inside its own image ----
            vs = sb.tile([P, BLOC * nch], f32)
            nc.vector.tensor_tensor(out=vs[:], in0=cum[:], in1=oh[:], op=AL.mult)
            rankf = sb.tile([P, nch], f32)
            nc.vector.tensor_reduce(
                out=rankf[:],
                in_=vs[:].rearrange("p (c j) -> p j c", j=nch),
                axis=mybir.AxisListType.X,
                op=AL.add,
            )

            # ---- scatter slot = b*300 + rank - 1, drops pushed OOB ----
            rank0 = sb.tile([P, nch], i32)
            nc.vector.tensor_scalar(
                out=rank0[:], in0=rankf[:], scalar1=-1.0, scalar2=None, op0=AL.add
            )
            slot = sb.tile([P, nch], i32)
            nc.vector.tensor_scalar(
                out=slot[:], in0=bg[:], scalar1=MAX_PRED, scalar2=None, op0=AL.mult
            )
            nc.vector.tensor_tensor(out=slot[:], in0=slot[:], in1=rank0[:], op=AL.add)
            ge = sb.tile([P, nch], i32)
            nc.vector.tensor_scalar(
                out=ge[:],
                in0=rankf[:],
                scalar1=float(MAX_PRED) + 0.5,
                scalar2=float(BIG),
                op0=AL.is_ge,
                op1=AL.mult,
            )
            iv = sb.tile([P, nch], i32)
            nc.vector.tensor_scalar(
                out=iv[:],
                in0=val_i,
                scalar1=0,
                scalar2=BIG,
                op0=AL.is_equal,
                op1=AL.mult,
            )
            nc.vector.tensor_tensor(out=slot[:], in0=slot[:], in1=ge[:], op=AL.add)
            nc.vector.tensor_tensor(out=slot[:], in0=slot[:], in1=iv[:], op=AL.add)

            # ---- assemble 6-wide rows: [x1 y1 x2 y2 score class] ----
            rows = sb.tile([P, 6 * nch], f32)
            rows3 = rows[:].rearrange("p (j s) -> p j s", s=6)
            nc.vector.tensor_copy(
                out=rows3[:, :, 0:4],
                in_=gath[:].rearrange("p (j s) -> p j s", s=TW)[:, :, 0:4],
            )
            nc.vector.tensor_copy(
                out=rows3[:, :, 4:5],
                in_=sco[:].rearrange("p (j s) -> p j s", s=1),
            )
            nc.vector.tensor_copy(
                out=rows3[:, :, 5:6],
                in_=lab_i.rearrange("p (j s) -> p j s", s=1),
            )

            # ---- scatters: independent per-chunk outputs (no WAW chain) ----
            for j in range(nch):
                nc.gpsimd.indirect_dma_start(
                    out=outs[j].ap(),
                    out_offset=bass.IndirectOffsetOnAxis(ap=slot[:, j : j + 1], axis=0),
                    in_=rows[:, 6 * j : 6 * j + 6],
                    in_offset=None,
                    bounds_check=NSLOT - 1,
                    oob_is_err=False,
                )

            # ---- per-image counts: reduce tot over chunks ----
            cntf = sb.tile([1, BLOC], f32)
            nc.vector.tensor_reduce(
                out=cntf[:],
                in_=tot[:].rearrange("p (c j) -> p c j", j=nch),
                axis=mybir.AxisListType.X,
                op=AL.add,
            )
            cnti = sb.tile([1, BLOC], i32)
            nc.vector.tensor_copy(out=cnti[:], in_=cntf[:])
            nc.sync.dma_start(num_pred.ap().rearrange("b one -> one b"), cnti[:])

    nc.compile()
    return nc


def get_nc(nch=NCH_DEFAULT):
    if nch not in _CACHE:
        _CACHE[nch] = _build(nch)
    return _CACHE[nch]


def _pack(arr, nch):
    """[nch*128] row-ordered -> [128, nch] with element [p, j] = row j*128+p."""
    return np.ascontiguousarray(arr.reshape(nch, P).T)


def shard_inputs(pred_boxes, pred_scores, selected_indexes, nch):
    """Build the 8 per-core input maps from the full inputs."""
    pred_boxes = np.asarray(pred_boxes, dtype=np.float32)
    pred_scores = np.asarray(pred_scores, dtype=np.float32)
    sel = np.asarray(selected_indexes).astype(np.int64)
    srows = nch * P

    b_all = sel[:, 0]
    in_maps = []
    for core in range(NCORES):
        rows = np.nonzero((b_all >= core * BLOC) & (b_all < (core + 1) * BLOC))[0]
        cnt = len(rows)
        assert cnt <= srows, f"core {core} has {cnt} rows > {srows}"
        bs = np.full(srows, BLOC, np.int32)
        labp = np.zeros(srows, np.int32)
        boxp = np.zeros(srows, np.int32)
        vld = np.zeros(srows, np.int32)
        bs[:cnt] = (sel[rows, 0] - core * BLOC).astype(np.int32)
        labp[:cnt] = sel[rows, 1].astype(np.int32)
        boxp[:cnt] = sel[rows, 2].astype(np.int32)
        vld[:cnt] = 1
        sidx = np.concatenate(
            [_pack(bs, nch), _pack(labp, nch), _pack(boxp, nch), _pack(vld, nch)],
            axis=1,
        )  # [P, 4*nch]
        tbl = np.concatenate(
            [
                pred_boxes[core * BLOC : (core + 1) * BLOC],
                pred_scores[core * BLOC : (core + 1) * BLOC],
            ],
            axis=2,
        ).reshape(BLOC * N, TW)
        in_maps.append(
            {
                "tbl": np.ascontiguousarray(tbl),
                "sidx": np.ascontiguousarray(sidx),
            }
        )
    return in_maps


def unshard_outputs(results, nch):
    """results: list of 8 dicts name->array. Returns the reference tuple."""
    num_pred = np.concatenate([r["num_pred"] for r in results], axis=0).astype(
        np.int32
    )  # [64, 1]
    rows = np.stack(
        [sum(r[f"out_rows_{j}"] for j in range(nch)) for r in results], axis=0
    )  # [8, 2400, 6]
    rows = rows.reshape(B, MAX_PRED, 6)
    out_boxes = np.ascontiguousarray(rows[:, :, 0:4], dtype=np.float32)
    out_scores = np.ascontiguousarray(rows[:, :, 4], dtype=np.float32)
    out_classes = rows[:, :, 5].astype(np.int32)
    return num_pred, out_boxes, out_scores, out_classes


def _pick_nch(selected_indexes):
    sel = np.asarray(selected_indexes).astype(np.int64)
    counts = np.bincount(sel[:, 0] // BLOC, minlength=NCORES)
    need = int(counts.max())
    nch = NCH_DEFAULT
    while nch * P < need:
        nch += 1
    return nch


def kernel(pred_boxes, pred_scores, selected_indexes):
    from concourse import bass_utils

    nch = _pick_nch(selected_indexes)
    nc = get_nc(nch)
    in_maps = shard_inputs(pred_boxes, pred_scores, selected_indexes, nch)
    res = bass_utils.run_bass_kernel_spmd(nc, in_maps, core_ids=list(range(NCORES)))
    return unshard_outputs(res.results, nch)
